# revision 1
# baseline (speedup 1.0000x reference)
"""Trainium2 Bass kernel for ContentPopularityJointAttention.

Computes, for each batch row b:
    mp     = concat(m[b], p[b])            # (50, 512)
    hidden = tanh(mp @ Wu)                 # (50, 512)
    s      = hidden @ bvec                 # (50,)
    u[b]   = (sum_n s_n * m[b,n]) / (sum_n s_n)   # (256,)

Sharding: pure data parallel over the batch dim across 8 NeuronCores.

Per-core dataflow (tokens = batch-rows*50 = 25600, processed in 128-token
chunks):
  1. DMA m,p chunk into one [128, 512] fp32 SBUF tile (token-major).
  2. 4 PE transposes -> PSUM [128(d), 512(tok-chunked)] fp32  (feature-major).
  3. fp16 hi/lo split of the transposed data (precision: the sum-normalized
     attention amplifies error ~1/|sum s|, so the hidden matmul needs
     ~fp32-grade products; a 3-term fp16 split reaches 4e-4 rel error
     at 3 cycles/row instead of fp32's 4).
  4. 12 fp16 matmuls: (hiT@Wu_hi + loT@Wu_hi + hiT@Wu_lo), Wu moving,
     mpT chunks stationary -> hidden [128(tok), 512] fp32 PSUM.
  5. ACT tanh -> SBUF fp32.
  6. DVE tensor_tensor_reduce with b replicated across partitions ->
     per-token scores s [128, 1] fp32 (products in fp32).
  7. DVE s * block-diagonal row mask -> lhsT [128, 68] fp16; one fp16
     pooling matmul with rhs = [m | ones] accumulates [sum s*m | sum s]
     into a 64-row group PSUM (rows of 50 tokens may straddle chunks;
     PSUM accumulation handles the overlap).
  8. Per 64-row group: DVE reciprocal + scale -> u rows, DMA out.
"""

import numpy as np
from contextlib import ExitStack

import concourse.bass as bass
import concourse.bacc as bacc
import concourse.tile as tile
from concourse import mybir
from concourse.bass_utils import run_bass_kernel_spmd

N_CORES = 8
B_FULL, N_TOK, MD, PD = 4096, 50, 256, 256
D = MD + PD          # 512 contraction dim
K = 512              # hidden dim
CHUNK = 128          # tokens per chunk (partition dim)
GROUP_ROWS = 64      # batch rows per pooling PSUM accumulation group
GROUP_CHUNKS = GROUP_ROWS * N_TOK // CHUNK   # 25
POOL_P = 68          # pooling PSUM partitions (max local row 63 + span 4)

f32 = mybir.dt.float32
f16 = mybir.dt.float16
bf16 = mybir.dt.bfloat16


def build_program(b_shard: int):
    """Build the single-core Bass program (SPMD: same program, all cores)."""
    tokens = b_shard * N_TOK
    assert tokens % (CHUNK * GROUP_CHUNKS) == 0
    n_groups = b_shard // GROUP_ROWS

    nc = bacc.Bacc("TRN2", target_bir_lowering=False, debug=False,
                   num_devices=N_CORES)

    m_d = nc.dram_tensor("m", [tokens, MD], f32, kind="ExternalInput").ap()
    p_d = nc.dram_tensor("p", [tokens, PD], f32, kind="ExternalInput").ap()
    wu_hi_d = nc.dram_tensor("wu_hi", [128, 4, K], f16, kind="ExternalInput").ap()
    wu_lo_d = nc.dram_tensor("wu_lo", [128, 4, K], f16, kind="ExternalInput").ap()
    brep_d = nc.dram_tensor("brep", [128, K], f32, kind="ExternalInput").ap()
    ident_d = nc.dram_tensor("ident", [128, 128], f32, kind="ExternalInput").ap()
    masks_d = nc.dram_tensor("masks", [128, GROUP_CHUNKS, POOL_P], f16,
                             kind="ExternalInput").ap()
    u_d = nc.dram_tensor("u", [b_shard, MD], f32, kind="ExternalOutput").ap()

    with tile.TileContext(nc) as tc, ExitStack() as ctx:
        singles = ctx.enter_context(tc.tile_pool(name="singles", bufs=1))
        io_pool = ctx.enter_context(tc.tile_pool(name="io", bufs=4))
        work = ctx.enter_context(tc.tile_pool(name="work", bufs=3))
        psum_t = ctx.enter_context(tc.tile_pool(name="psumT", bufs=2, space="PSUM"))
        psum_h = ctx.enter_context(tc.tile_pool(name="psumH", bufs=2, space="PSUM"))
        psum_u = ctx.enter_context(tc.tile_pool(name="psumU", bufs=2, space="PSUM"))

        wu_hi_sb = singles.tile([128, 4, K], f16)
        nc.gpsimd.dma_start(out=wu_hi_sb[:], in_=wu_hi_d)
        wu_lo_sb = singles.tile([128, 4, K], f16)
        nc.gpsimd.dma_start(out=wu_lo_sb[:], in_=wu_lo_d)
        brep_sb = singles.tile([128, K], f32)
        nc.gpsimd.dma_start(out=brep_sb[:], in_=brep_d)
        ident_sb = singles.tile([128, 128], f32)
        nc.gpsimd.dma_start(out=ident_sb[:], in_=ident_d)
        masks_sb = singles.tile([128, GROUP_CHUNKS, POOL_P], f16)
        nc.gpsimd.dma_start(out=masks_sb[:], in_=masks_d)

        for g in range(n_groups):
            pool_ps = psum_u.tile([POOL_P, MD + 1], f32)
            for l in range(GROUP_CHUNKS):
                c = g * GROUP_CHUNKS + l
                t0 = c * CHUNK

                mp32 = io_pool.tile([128, D], f32)
                nc.gpsimd.dma_start(out=mp32[:, 0:MD], in_=m_d[t0:t0 + CHUNK, :])
                nc.gpsimd.dma_start(out=mp32[:, MD:D], in_=p_d[t0:t0 + CHUNK, :])

                # transpose to feature-major
                psT = psum_t.tile([128, D], f32)
                for j in range(4):
                    nc.tensor.transpose(
                        psT[:, j * 128:(j + 1) * 128],
                        mp32[:, j * 128:(j + 1) * 128],
                        ident_sb[:],
                    )

                # fp16 hi/lo split (in transposed domain)
                mpT_hi = work.tile([128, D], f16)
                nc.scalar.copy(out=mpT_hi[:], in_=psT[:])
                mpT_hi32 = work.tile([128, D], f32)
                nc.gpsimd.tensor_copy(out=mpT_hi32[:], in_=mpT_hi[:])
                mpT_lo = work.tile([128, D], f16)
                nc.vector.tensor_sub(mpT_lo[:], psT[:], mpT_hi32[:])

                # hidden = tanh(mp @ Wu), 3-term fp16 split
                hid = psum_h.tile([128, K], f32)
                n_mm = 12
                i_mm = 0
                for lhs in (mpT_hi, mpT_lo):
                    for j in range(4):
                        nc.tensor.matmul(
                            hid[:],
                            lhsT=lhs[:, j * 128:(j + 1) * 128],
                            rhs=wu_hi_sb[:, j, :],
                            start=(i_mm == 0),
                            stop=(i_mm == n_mm - 1),
                        )
                        i_mm += 1
                for j in range(4):
                    nc.tensor.matmul(
                        hid[:],
                        lhsT=mpT_hi[:, j * 128:(j + 1) * 128],
                        rhs=wu_lo_sb[:, j, :],
                        start=(i_mm == 0),
                        stop=(i_mm == n_mm - 1),
                    )
                    i_mm += 1

                tanhH = work.tile([128, K], f32)
                nc.scalar.activation(out=tanhH[:], in_=hid[:],
                                     func=mybir.ActivationFunctionType.Tanh)

                # s[tok] = sum_k tanhH * b   (fp32 products on DVE;
                # tensor_tensor_reduce crashes NRT in this env, use two ops)
                scr = work.tile([128, K], f32)
                s = work.tile([128, 1], f32)
                nc.vector.tensor_mul(scr[:], tanhH[:], brep_sb[:])
                nc.vector.reduce_sum(s[:], scr[:], axis=mybir.AxisListType.X)

                # block-diagonal pooling lhsT and [m | 1] rhs (fp32: the
                # ones-column sum S is cancellation-amplified, fp16 is not
                # enough there)
                blk = work.tile([128, POOL_P], f32)
                nc.vector.tensor_scalar_mul(blk[:], masks_sb[:, l, :], s[:])
                m16 = work.tile([128, MD + 1], f32)
                nc.gpsimd.tensor_copy(out=m16[:, 0:MD], in_=mp32[:, 0:MD])
                nc.vector.memset(m16[:, MD:MD + 1], 1.0)
                nc.tensor.matmul(
                    pool_ps[:],
                    lhsT=blk[:],
                    rhs=m16[:],
                    start=(l == 0),
                    stop=(l == GROUP_CHUNKS - 1),
                )

            rS = work.tile([GROUP_ROWS, 1], f32)
            nc.vector.reciprocal(rS[:], pool_ps[0:GROUP_ROWS, MD:MD + 1])
            u_sb = io_pool.tile([GROUP_ROWS, MD], f32)
            nc.vector.tensor_scalar_mul(u_sb[:], pool_ps[0:GROUP_ROWS, 0:MD], rS[:])
            nc.gpsimd.dma_start(
                out=u_d[g * GROUP_ROWS:(g + 1) * GROUP_ROWS, :], in_=u_sb[:])

    nc.compile()
    return nc


def host_constants(Wu: np.ndarray, b: np.ndarray):
    Wu = np.asarray(Wu, np.float32)
    b = np.asarray(b, np.float32)
    wu_hi16 = Wu.astype(np.float16)
    wu_lo16 = (Wu - wu_hi16.astype(np.float32)).astype(np.float16)
    # [d, k] -> [d%128, d//128, k]
    wu_hi = np.ascontiguousarray(wu_hi16.reshape(4, 128, K).transpose(1, 0, 2))
    wu_lo = np.ascontiguousarray(wu_lo16.reshape(4, 128, K).transpose(1, 0, 2))
    brep = np.ascontiguousarray(np.broadcast_to(b, (128, K)))
    ident = np.eye(128, dtype=np.float32)
    tp = np.arange(128)[:, None, None]
    ll = np.arange(GROUP_CHUNKS)[None, :, None]
    rr = np.arange(POOL_P)[None, None, :]
    masks = (((CHUNK * ll + tp) // N_TOK) == rr).astype(np.float16)
    return {"wu_hi": wu_hi, "wu_lo": wu_lo, "brep": brep, "ident": ident,
            "masks": masks}


_prog_cache: dict = {}


def get_program(b_shard: int):
    if b_shard not in _prog_cache:
        _prog_cache[b_shard] = build_program(b_shard)
    return _prog_cache[b_shard]


def kernel(m: np.ndarray, p: np.ndarray, Wu: np.ndarray, b: np.ndarray
           ) -> np.ndarray:
    m = np.ascontiguousarray(np.asarray(m, np.float32))
    p = np.ascontiguousarray(np.asarray(p, np.float32))
    B = m.shape[0]
    assert B % N_CORES == 0
    b_shard = B // N_CORES

    nc = get_program(b_shard)
    consts = host_constants(Wu, b)

    mf = m.reshape(B * N_TOK, MD)
    pf = p.reshape(B * N_TOK, PD)
    tok_sh = b_shard * N_TOK
    in_maps = []
    for c in range(N_CORES):
        in_maps.append({
            "m": mf[c * tok_sh:(c + 1) * tok_sh],
            "p": pf[c * tok_sh:(c + 1) * tok_sh],
            **consts,
        })
    res = run_bass_kernel_spmd(nc, in_maps, list(range(N_CORES)))
    u = np.concatenate([res.results[c]["u"] for c in range(N_CORES)], axis=0)
    return u.astype(np.float32)



# revision 4
# speedup vs baseline: 2.0790x; 2.0790x over previous
"""Trainium2 Bass kernel for ContentPopularityJointAttention.

Computes, for each batch row b:
    mp     = concat(m[b], p[b])            # (50, 512)
    hidden = tanh(mp @ Wu)                 # (50, 512)
    s      = hidden @ bvec                 # (50,)
    u[b]   = (sum_n s_n * m[b,n]) / (sum_n s_n)   # (256,)

Sharding: pure data parallel over the batch dim across 8 NeuronCores.

Precision: the sum-normalized attention amplifies score error by
~1/|sum s| (rel err ~ 60x the hidden-matmul abs error), so the hidden
matmul needs >=17-bit-effective products. Scheme (measured on HW):
  T1 = mp_hi16 @ Wu_hi16           4 fp16 matmuls        (full scale)
  T2 = mp_lo @ Wu_hi, T3 = mp_hi @ Wu_lo: both scaled by 2^12 and
       computed as fp8e4m3 DoubleRow matmuls (0.5 cyc/row, 4 matmuls
       over a stacked 1024-row contraction), accumulated in a second
       PSUM, then folded into T1's PSUM via an fp16 identity matmul
       after a 2^-12-scaled fp16 copy on ACT.
All splits are done on the host; inputs arrive pre-transposed
feature-major so no PE transposes are needed.

Per-chunk (128 tokens) engine budget (CoreSim cost model):
  PE  ~1600ns (4 DR + 4 fp16 + combine + 2 pooling matmuls)  <- bound
  ACT ~1224ns (scaled fp8-term copy + tanh)
  Pool ~1111ns (score elementwise mul th*b)
  DVE  ~840ns (score reduce, mask scaling, group finalize)
  DMA  ~910ns (hi16 + stacked fp8 + token-major m16)
"""

import numpy as np
import ml_dtypes
from contextlib import ExitStack

import concourse.bass as bass
import concourse.bacc as bacc
import concourse.tile as tile
from concourse import mybir
from concourse.bass_utils import run_bass_kernel_spmd

N_CORES = 8
B_FULL, N_TOK, MD, PD = 4096, 50, 256, 256
D = MD + PD          # 512 contraction dim
K = 512              # hidden dim
CHUNK = 128          # tokens per chunk
GROUP_ROWS = 64      # batch rows per pooling group
GROUP_CHUNKS = GROUP_ROWS * N_TOK // CHUNK   # 25
POOL_P = 64
SUPER = 16           # chunks per input-DMA superchunk
LO_SCALE = 4096.0    # 2^12 scale for the fp8 correction terms

f32 = mybir.dt.float32
f16 = mybir.dt.float16
f8 = mybir.dt.float8e4
np_f8 = ml_dtypes.float8_e4m3


def build_program(b_shard: int):
    """Build the single-core Bass program (SPMD: same program, all cores)."""
    tokens = b_shard * N_TOK
    n_chunks = tokens // CHUNK
    assert b_shard % GROUP_ROWS == 0
    n_supers = (n_chunks + SUPER - 1) // SUPER

    nc = bacc.Bacc("TRN2", target_bir_lowering=False, debug=False,
                   num_devices=N_CORES)

    hi16_d = nc.dram_tensor("hi16", [128, 4, tokens], f16,
                            kind="ExternalInput").ap()
    lo8_d = nc.dram_tensor("lo8", [128, 4, 2, tokens], f8,
                           kind="ExternalInput").ap()
    m16_d = nc.dram_tensor("m16", [128, n_chunks, MD], f16,
                           kind="ExternalInput").ap()
    wu16_d = nc.dram_tensor("wu16", [128, 4, K], f16, kind="ExternalInput").ap()
    wu8_d = nc.dram_tensor("wu8", [128, 4, 2, K], f8, kind="ExternalInput").ap()
    brep_d = nc.dram_tensor("brep", [128, K], f32, kind="ExternalInput").ap()
    masks_d = nc.dram_tensor("masks", [128, GROUP_CHUNKS, POOL_P], f16,
                             kind="ExternalInput").ap()
    ones_d = nc.dram_tensor("ones", [128, 1], f32, kind="ExternalInput").ap()
    ident_d = nc.dram_tensor("ident", [128, 128], f16,
                             kind="ExternalInput").ap()
    u_d = nc.dram_tensor("u", [b_shard, MD], f32, kind="ExternalOutput").ap()

    with tile.TileContext(nc) as tc, ExitStack() as ctx:
        singles = ctx.enter_context(tc.tile_pool(name="singles", bufs=1))
        hi_pool = ctx.enter_context(tc.tile_pool(name="hi", bufs=2))
        lo_pool = ctx.enter_context(tc.tile_pool(name="lo", bufs=2))
        m_pool = ctx.enter_context(tc.tile_pool(name="m16", bufs=2))
        th_pool = ctx.enter_context(tc.tile_pool(name="th", bufs=2))
        scr_pool = ctx.enter_context(tc.tile_pool(name="scr", bufs=2))
        sb2_pool = ctx.enter_context(tc.tile_pool(name="sb2", bufs=2))
        small = ctx.enter_context(tc.tile_pool(name="small", bufs=3))
        usb_pool = ctx.enter_context(tc.tile_pool(name="usb", bufs=2))
        psum1 = ctx.enter_context(tc.tile_pool(name="psum1", bufs=2,
                                               space="PSUM"))
        psum2 = ctx.enter_context(tc.tile_pool(name="psum2", bufs=2,
                                               space="PSUM"))
        psum_u = ctx.enter_context(tc.tile_pool(name="psumU", bufs=2,
                                                space="PSUM"))
        psum_s = ctx.enter_context(tc.tile_pool(name="psumS", bufs=2,
                                                space="PSUM"))

        wu16_sb = singles.tile([128, 4, K], f16)
        nc.gpsimd.dma_start(out=wu16_sb[:], in_=wu16_d)
        wu8_sb = singles.tile([128, 4, 2, K], f8)
        nc.gpsimd.dma_start(out=wu8_sb[:], in_=wu8_d)
        brep_sb = singles.tile([128, K], f32)
        nc.gpsimd.dma_start(out=brep_sb[:], in_=brep_d)
        masks_sb = singles.tile([128, GROUP_CHUNKS, POOL_P], f16)
        nc.gpsimd.dma_start(out=masks_sb[:], in_=masks_d)
        ones_sb = singles.tile([128, 1], f32)
        nc.gpsimd.dma_start(out=ones_sb[:], in_=ones_d)
        ident_sb = singles.tile([128, 128], f16)
        nc.gpsimd.dma_start(out=ident_sb[:], in_=ident_d)

        pool_ps = None
        s_ps = None
        for sc in range(n_supers):
            nck = min(SUPER, n_chunks - sc * SUPER)
            t0 = sc * SUPER * CHUNK
            hi_sb = hi_pool.tile([128, 4, SUPER * CHUNK], f16)
            nc.sync.dma_start(out=hi_sb[:, :, 0:nck * CHUNK],
                              in_=hi16_d[:, :, t0:t0 + nck * CHUNK])
            lo_sb = lo_pool.tile([128, 4, 2, SUPER * CHUNK], f8)
            nc.sync.dma_start(out=lo_sb[:, :, :, 0:nck * CHUNK],
                              in_=lo8_d[:, :, :, t0:t0 + nck * CHUNK])
            m16_sb = m_pool.tile([128, SUPER, MD], f16)
            nc.sync.dma_start(out=m16_sb[:, 0:nck, :],
                              in_=m16_d[:, sc * SUPER:sc * SUPER + nck, :])

            for o in range(nck):
                c = sc * SUPER + o
                l = c % GROUP_CHUNKS
                g = c // GROUP_CHUNKS
                ts = o * CHUNK

                # fp8 DoubleRow correction terms (scaled by 2^12)
                p2 = psum2.tile([128, K], f32)
                for blk in range(4):
                    nc.tensor.matmul(
                        p2[:],
                        lhsT=lo_sb[:, blk, :, ts:ts + CHUNK],
                        rhs=wu8_sb[:, blk, :, :],
                        start=(blk == 0),
                        stop=(blk == 3),
                        perf_mode=mybir.MatmulPerfMode.DoubleRow,
                    )
                sb2 = sb2_pool.tile([128, K], f16)
                nc.scalar.mul(sb2[:], p2[:], 1.0 / LO_SCALE)

                # fp16 main term + fold-in of the correction
                p1 = psum1.tile([128, K], f32)
                for j in range(4):
                    nc.tensor.matmul(
                        p1[:],
                        lhsT=hi_sb[:, j, ts:ts + CHUNK],
                        rhs=wu16_sb[:, j, :],
                        start=(j == 0),
                        stop=False,
                    )
                nc.tensor.matmul(p1[:], lhsT=ident_sb[:], rhs=sb2[:],
                                 start=False, stop=True)

                th = th_pool.tile([128, K], f32)
                nc.scalar.activation(out=th[:], in_=p1[:],
                                     func=mybir.ActivationFunctionType.Tanh)

                # scores: s = (th * b).sum(k); products in fp32
                scr = scr_pool.tile([128, K], f32)
                nc.gpsimd.tensor_mul(scr[:], th[:], brep_sb[:])
                s = small.tile([128, 1], f32)
                nc.vector.reduce_sum(s[:], scr[:], axis=mybir.AxisListType.X)

                # block-diagonal pooling: fp16 numerator, fp32 S column
                blk16 = small.tile([128, POOL_P], f16)
                nc.vector.tensor_scalar_mul(blk16[:], masks_sb[:, l, :], s[:])
                blk32 = small.tile([128, POOL_P], f32)
                nc.vector.tensor_scalar_mul(blk32[:], masks_sb[:, l, :], s[:])

                if l == 0:
                    pool_ps = psum_u.tile([POOL_P, MD], f32)
                    s_ps = psum_s.tile([POOL_P, 1], f32)
                nc.tensor.matmul(
                    pool_ps[:],
                    lhsT=blk16[:],
                    rhs=m16_sb[:, o, :],
                    start=(l == 0),
                    stop=(l == GROUP_CHUNKS - 1),
                )
                nc.tensor.matmul(
                    s_ps[:],
                    lhsT=blk32[:],
                    rhs=ones_sb[:],
                    start=(l == 0),
                    stop=(l == GROUP_CHUNKS - 1),
                )

                if l == GROUP_CHUNKS - 1:
                    rec = small.tile([GROUP_ROWS, 1], f32)
                    nc.vector.reciprocal(rec[:], s_ps[0:GROUP_ROWS, :])
                    u_sb = usb_pool.tile([GROUP_ROWS, MD], f32)
                    nc.vector.tensor_scalar_mul(
                        u_sb[:], pool_ps[0:GROUP_ROWS, :], rec[:])
                    nc.sync.dma_start(
                        out=u_d[g * GROUP_ROWS:(g + 1) * GROUP_ROWS, :],
                        in_=u_sb[:])

    nc.compile()
    return nc


def host_constants(Wu: np.ndarray, b: np.ndarray):
    Wu = np.asarray(Wu, np.float32)
    b = np.asarray(b, np.float32)
    wu_hi16 = Wu.astype(np.float16)
    wu_hi32 = wu_hi16.astype(np.float32)
    wu_lo = Wu - wu_hi32
    # [d, k] -> [d%128, d//128, k]
    wu16 = np.ascontiguousarray(wu_hi16.reshape(4, 128, K).transpose(1, 0, 2))
    # stacked fp8 DoubleRow rhs: contraction row cx = 256*blk + 2*p + r;
    # cx<512 -> fp8(Wu_hi), cx>=512 -> fp8(Wu_lo * 2^12)
    WH = np.concatenate([wu_hi32.astype(np_f8),
                         (wu_lo * LO_SCALE).astype(np_f8)], axis=0)  # [1024,K]
    wu8 = np.ascontiguousarray(
        WH.reshape(4, 128, 2, K).transpose(1, 0, 2, 3))
    brep = np.ascontiguousarray(np.broadcast_to(b, (128, K)))
    tp = np.arange(128)[:, None, None]
    ll = np.arange(GROUP_CHUNKS)[None, :, None]
    rr = np.arange(POOL_P)[None, None, :]
    masks = (((CHUNK * ll + tp) // N_TOK) == rr).astype(np.float16)
    ones = np.ones((128, 1), np.float32)
    ident = np.eye(128, dtype=np.float16)
    return {"wu16": wu16, "wu8": wu8, "brep": brep, "masks": masks,
            "ones": ones, "ident": ident}


def host_inputs(m: np.ndarray, p: np.ndarray):
    """Feature-major fp16 hi + stacked fp8 lo/hi + token-major fp16 m."""
    B = m.shape[0]
    T = B * N_TOK
    mp = np.concatenate([m.reshape(T, MD), p.reshape(T, PD)], axis=1)  # [T,512]
    mp_hi16 = mp.astype(np.float16)
    mp_hi32 = mp_hi16.astype(np.float32)
    mp_lo = mp - mp_hi32
    hi16 = np.ascontiguousarray(
        mp_hi16.T.reshape(4, 128, T).transpose(1, 0, 2))         # [128,4,T]
    LH = np.concatenate([(mp_lo.T * LO_SCALE).astype(np_f8),
                         mp_hi32.T.astype(np_f8)], axis=0)       # [1024,T]
    lo8 = np.ascontiguousarray(
        LH.reshape(4, 128, 2, T).transpose(1, 0, 2, 3))          # [128,4,2,T]
    m16 = m.reshape(T, MD).astype(np.float16)
    m16 = np.ascontiguousarray(
        m16.reshape(-1, 128, MD).transpose(1, 0, 2))             # [128,C,MD]
    return hi16, lo8, m16


_prog_cache: dict = {}


def get_program(b_shard: int):
    if b_shard not in _prog_cache:
        _prog_cache[b_shard] = build_program(b_shard)
    return _prog_cache[b_shard]


def kernel(m: np.ndarray, p: np.ndarray, Wu: np.ndarray, b: np.ndarray
           ) -> np.ndarray:
    m = np.ascontiguousarray(np.asarray(m, np.float32))
    p = np.ascontiguousarray(np.asarray(p, np.float32))
    B = m.shape[0]
    assert B % N_CORES == 0
    b_shard = B // N_CORES

    nc = get_program(b_shard)
    consts = host_constants(Wu, b)
    hi16, lo8, m16 = host_inputs(m, p)

    tok_sh = b_shard * N_TOK
    chk_sh = tok_sh // CHUNK
    in_maps = []
    for c in range(N_CORES):
        tsl = slice(c * tok_sh, (c + 1) * tok_sh)
        csl = slice(c * chk_sh, (c + 1) * chk_sh)
        in_maps.append({
            "hi16": np.ascontiguousarray(hi16[:, :, tsl]),
            "lo8": np.ascontiguousarray(lo8[:, :, :, tsl]),
            "m16": np.ascontiguousarray(m16[:, csl, :]),
            **consts,
        })
    res = run_bass_kernel_spmd(nc, in_maps, list(range(N_CORES)))
    u = np.concatenate([res.results[c]["u"] for c in range(N_CORES)], axis=0)
    return u.astype(np.float32)


# revision 12
# speedup vs baseline: 2.3504x; 1.1305x over previous
"""Trainium2 Bass kernel for ContentPopularityJointAttention.

Computes, for each batch row b:
    mp     = concat(m[b], p[b])            # (50, 512)
    hidden = tanh(mp @ Wu)                 # (50, 512)
    s      = hidden @ bvec                 # (50,)
    u[b]   = (sum_n s_n * m[b,n]) / (sum_n s_n)   # (256,)

Sharding: pure data parallel over the batch dim across 8 NeuronCores.

Precision: the sum-normalized attention amplifies score error by
~1/|sum s| (rel err ~ 60x the hidden-matmul abs error), so the hidden
matmul needs >=17-bit-effective products. Scheme (measured on HW):
  T1 = (mp_hi16*2^6) @ (Wu_hi16*2^6)   4 fp16 matmuls   (2^6 exact)
  T2 = (mp_lo*2^12) @ Wu_hi, T3 = mp_hi @ (Wu_lo*2^12): fp8e4m3
       DoubleRow matmuls (0.5 cyc/row, 4 matmuls over a stacked
       1024-row contraction).
All three terms carry the same 2^12 scale, so they accumulate in ONE
PSUM; the descale rides for free on tanh's activation `scale`.
All splits are done on the host; inputs arrive pre-transposed
feature-major so no PE transposes are needed.

Per-chunk (128 tokens) engine budget (CoreSim cost model):
  PE  ~1390ns (4 DR + 4 fp16 + 2 pooling matmuls)  <- bound
  Pool ~1111ns (score elementwise mul th*b)
  DVE  ~840ns (score reduce, mask scaling, group finalize)
  DMA  ~910ns (hi16 + stacked fp8 + token-major m16)
  ACT  ~612ns (tanh with 2^-12 descale)
"""

import numpy as np
import ml_dtypes
from contextlib import ExitStack

import concourse.bass as bass
import concourse.bacc as bacc
import concourse.tile as tile
from concourse import mybir
from concourse.bass_utils import run_bass_kernel_spmd

N_CORES = 8
B_FULL, N_TOK, MD, PD = 4096, 50, 256, 256
D = MD + PD          # 512 contraction dim
K = 512              # hidden dim
CHUNK = 128          # tokens per chunk
GROUP_ROWS = 64      # batch rows per pooling group
GROUP_CHUNKS = GROUP_ROWS * N_TOK // CHUNK   # 25
POOL_P = 64
SUPER = 16           # chunks per input-DMA superchunk
LO_SCALE = 4096.0    # 2^12 scale for the fp8 correction terms

f32 = mybir.dt.float32
f16 = mybir.dt.float16
f8 = mybir.dt.float8e4
np_f8 = ml_dtypes.float8_e4m3


def build_program(b_shard: int):
    """Build the single-core Bass program (SPMD: same program, all cores)."""
    tokens = b_shard * N_TOK
    n_chunks = tokens // CHUNK
    assert b_shard % GROUP_ROWS == 0
    n_supers = (n_chunks + SUPER - 1) // SUPER

    nc = bacc.Bacc("TRN2", target_bir_lowering=False, debug=False,
                   num_devices=N_CORES)

    hi16_d = nc.dram_tensor("hi16", [128, 4, tokens], f16,
                            kind="ExternalInput").ap()
    lo8_d = nc.dram_tensor("lo8", [128, 4, 2, tokens], f8,
                           kind="ExternalInput").ap()
    m16_d = nc.dram_tensor("m16", [128, n_chunks, MD], f16,
                           kind="ExternalInput").ap()
    wu16_d = nc.dram_tensor("wu16", [128, 4, K], f16, kind="ExternalInput").ap()
    wu8_d = nc.dram_tensor("wu8", [128, 4, 2, K], f8, kind="ExternalInput").ap()
    brep_d = nc.dram_tensor("brep", [128, K], f32, kind="ExternalInput").ap()
    masks_d = nc.dram_tensor("masks", [128, GROUP_CHUNKS, POOL_P], f16,
                             kind="ExternalInput").ap()
    ones_d = nc.dram_tensor("ones", [128, 1], f32, kind="ExternalInput").ap()
    u_d = nc.dram_tensor("u", [b_shard, MD], f32, kind="ExternalOutput").ap()

    with tile.TileContext(nc) as tc, ExitStack() as ctx:
        singles = ctx.enter_context(tc.tile_pool(name="singles", bufs=1))
        hi_pool = ctx.enter_context(tc.tile_pool(name="hi", bufs=2))
        lo_pool = ctx.enter_context(tc.tile_pool(name="lo", bufs=2))
        m_pool = ctx.enter_context(tc.tile_pool(name="m16", bufs=2))
        th_pool = ctx.enter_context(tc.tile_pool(name="th", bufs=2))
        scr_pool = ctx.enter_context(tc.tile_pool(name="scr", bufs=2))
        small = ctx.enter_context(tc.tile_pool(name="small", bufs=3))
        usb_pool = ctx.enter_context(tc.tile_pool(name="usb", bufs=2))
        psum1 = ctx.enter_context(tc.tile_pool(name="psum1", bufs=3,
                                               space="PSUM"))
        psum_u = ctx.enter_context(tc.tile_pool(name="psumU", bufs=2,
                                                space="PSUM"))
        psum_s = ctx.enter_context(tc.tile_pool(name="psumS", bufs=2,
                                                space="PSUM"))

        wu16_sb = singles.tile([128, 4, K], f16)
        nc.gpsimd.dma_start(out=wu16_sb[:], in_=wu16_d)
        wu8_sb = singles.tile([128, 4, 2, K], f8)
        nc.gpsimd.dma_start(out=wu8_sb[:], in_=wu8_d)
        brep_sb = singles.tile([128, K], f32)
        nc.gpsimd.dma_start(out=brep_sb[:], in_=brep_d)
        masks_sb = singles.tile([128, GROUP_CHUNKS, POOL_P], f16)
        nc.gpsimd.dma_start(out=masks_sb[:], in_=masks_d)
        ones_sb = singles.tile([128, 1], f32)
        nc.gpsimd.dma_start(out=ones_sb[:], in_=ones_d)

        pool_ps = None
        s_ps = None
        for sc in range(n_supers):
            nck = min(SUPER, n_chunks - sc * SUPER)
            t0 = sc * SUPER * CHUNK
            hi_sb = hi_pool.tile([128, 4, SUPER * CHUNK], f16)
            nc.sync.dma_start(out=hi_sb[:, :, 0:nck * CHUNK],
                              in_=hi16_d[:, :, t0:t0 + nck * CHUNK])
            lo_sb = lo_pool.tile([128, 4, 2, SUPER * CHUNK], f8)
            nc.sync.dma_start(out=lo_sb[:, :, :, 0:nck * CHUNK],
                              in_=lo8_d[:, :, :, t0:t0 + nck * CHUNK])
            m16_sb = m_pool.tile([128, SUPER, MD], f16)
            nc.sync.dma_start(out=m16_sb[:, 0:nck, :],
                              in_=m16_d[:, sc * SUPER:sc * SUPER + nck, :])

            for o in range(nck):
                c = sc * SUPER + o
                l = c % GROUP_CHUNKS
                g = c // GROUP_CHUNKS
                ts = o * CHUNK

                # all three terms at 2^12 scale into one PSUM:
                # fp8 DoubleRow corrections + fp16 main term (2^6-scaled ops)
                p1 = psum1.tile([128, K], f32)
                for blk in range(4):
                    nc.tensor.matmul(
                        p1[:],
                        lhsT=lo_sb[:, blk, :, ts:ts + CHUNK],
                        rhs=wu8_sb[:, blk, :, :],
                        start=(blk == 0),
                        stop=False,
                        perf_mode=mybir.MatmulPerfMode.DoubleRow,
                    )
                for j in range(4):
                    nc.tensor.matmul(
                        p1[:],
                        lhsT=hi_sb[:, j, ts:ts + CHUNK],
                        rhs=wu16_sb[:, j, :],
                        start=False,
                        stop=(j == 3),
                    )

                th = th_pool.tile([128, K], f32)
                nc.scalar.activation(out=th[:], in_=p1[:],
                                     func=mybir.ActivationFunctionType.Tanh,
                                     scale=1.0 / LO_SCALE)

                # scores: s = (th * b).sum(k); products in fp32
                scr = scr_pool.tile([128, K], f32)
                nc.gpsimd.tensor_mul(scr[:], th[:], brep_sb[:])
                s = small.tile([128, 1], f32)
                nc.vector.reduce_sum(s[:], scr[:], axis=mybir.AxisListType.X)

                # block-diagonal pooling: fp16 numerator, fp32 S column
                blk16 = small.tile([128, POOL_P], f16)
                nc.vector.tensor_scalar_mul(blk16[:], masks_sb[:, l, :], s[:])
                blk32 = small.tile([128, POOL_P], f32)
                nc.vector.tensor_scalar_mul(blk32[:], masks_sb[:, l, :], s[:])

                if l == 0:
                    pool_ps = psum_u.tile([POOL_P, MD], f32)
                    s_ps = psum_s.tile([POOL_P, 1], f32)
                nc.tensor.matmul(
                    pool_ps[:],
                    lhsT=blk16[:],
                    rhs=m16_sb[:, o, :],
                    start=(l == 0),
                    stop=(l == GROUP_CHUNKS - 1),
                )
                nc.tensor.matmul(
                    s_ps[:],
                    lhsT=blk32[:],
                    rhs=ones_sb[:],
                    start=(l == 0),
                    stop=(l == GROUP_CHUNKS - 1),
                )

                if l == GROUP_CHUNKS - 1:
                    rec = small.tile([GROUP_ROWS, 1], f32)
                    nc.vector.reciprocal(rec[:], s_ps[0:GROUP_ROWS, :])
                    u_sb = usb_pool.tile([GROUP_ROWS, MD], f32)
                    nc.vector.tensor_scalar_mul(
                        u_sb[:], pool_ps[0:GROUP_ROWS, :], rec[:])
                    nc.sync.dma_start(
                        out=u_d[g * GROUP_ROWS:(g + 1) * GROUP_ROWS, :],
                        in_=u_sb[:])

    nc.compile()
    return nc


def host_constants(Wu: np.ndarray, b: np.ndarray):
    Wu = np.asarray(Wu, np.float32)
    b = np.asarray(b, np.float32)
    wu_hi16 = Wu.astype(np.float16)
    wu_hi32 = wu_hi16.astype(np.float32)
    wu_lo = Wu - wu_hi32
    # [d, k] -> [d%128, d//128, k]; 2^6 scale is exact in fp16
    wu16 = np.ascontiguousarray(
        (wu_hi32 * 64.0).astype(np.float16).reshape(4, 128, K)
        .transpose(1, 0, 2))
    # stacked fp8 DoubleRow rhs: contraction row cx = 256*blk + 2*p + r;
    # cx<512 -> fp8(Wu_hi), cx>=512 -> fp8(Wu_lo * 2^12)
    WH = np.concatenate([wu_hi32.astype(np_f8),
                         (wu_lo * LO_SCALE).astype(np_f8)], axis=0)  # [1024,K]
    wu8 = np.ascontiguousarray(
        WH.reshape(4, 128, 2, K).transpose(1, 0, 2, 3))
    brep = np.ascontiguousarray(np.broadcast_to(b, (128, K)))
    tp = np.arange(128)[:, None, None]
    ll = np.arange(GROUP_CHUNKS)[None, :, None]
    rr = np.arange(POOL_P)[None, None, :]
    masks = (((CHUNK * ll + tp) // N_TOK) == rr).astype(np.float16)
    ones = np.ones((128, 1), np.float32)
    return {"wu16": wu16, "wu8": wu8, "brep": brep, "masks": masks,
            "ones": ones}


def host_inputs(m: np.ndarray, p: np.ndarray):
    """Feature-major fp16 hi + stacked fp8 lo/hi + token-major fp16 m."""
    B = m.shape[0]
    T = B * N_TOK
    mp = np.concatenate([m.reshape(T, MD), p.reshape(T, PD)], axis=1)  # [T,512]
    mp_hi16 = mp.astype(np.float16)
    mp_hi32 = mp_hi16.astype(np.float32)
    mp_lo = mp - mp_hi32
    hi16 = np.ascontiguousarray(
        (mp_hi32 * 64.0).astype(np.float16).T
        .reshape(4, 128, T).transpose(1, 0, 2))                  # [128,4,T]
    LH = np.concatenate([(mp_lo.T * LO_SCALE).astype(np_f8),
                         mp_hi32.T.astype(np_f8)], axis=0)       # [1024,T]
    lo8 = np.ascontiguousarray(
        LH.reshape(4, 128, 2, T).transpose(1, 0, 2, 3))          # [128,4,2,T]
    m16 = m.reshape(T, MD).astype(np.float16)
    m16 = np.ascontiguousarray(
        m16.reshape(-1, 128, MD).transpose(1, 0, 2))             # [128,C,MD]
    return hi16, lo8, m16


_prog_cache: dict = {}


def get_program(b_shard: int):
    if b_shard not in _prog_cache:
        _prog_cache[b_shard] = build_program(b_shard)
    return _prog_cache[b_shard]


def kernel(m: np.ndarray, p: np.ndarray, Wu: np.ndarray, b: np.ndarray
           ) -> np.ndarray:
    m = np.ascontiguousarray(np.asarray(m, np.float32))
    p = np.ascontiguousarray(np.asarray(p, np.float32))
    B = m.shape[0]
    assert B % N_CORES == 0
    b_shard = B // N_CORES

    nc = get_program(b_shard)
    consts = host_constants(Wu, b)
    hi16, lo8, m16 = host_inputs(m, p)

    tok_sh = b_shard * N_TOK
    chk_sh = tok_sh // CHUNK
    in_maps = []
    for c in range(N_CORES):
        tsl = slice(c * tok_sh, (c + 1) * tok_sh)
        csl = slice(c * chk_sh, (c + 1) * chk_sh)
        in_maps.append({
            "hi16": np.ascontiguousarray(hi16[:, :, tsl]),
            "lo8": np.ascontiguousarray(lo8[:, :, :, tsl]),
            "m16": np.ascontiguousarray(m16[:, csl, :]),
            **consts,
        })
    res = run_bass_kernel_spmd(nc, in_maps, list(range(N_CORES)))
    u = np.concatenate([res.results[c]["u"] for c in range(N_CORES)], axis=0)
    return u.astype(np.float32)
